# revision 47
# baseline (speedup 1.0000x reference)
"""Multi-head attention (B=4, T=2048, D=1024, H=16, Dh=64) on 8 trn2 cores.

Sharding: core c handles batch b = c//2 and head-half hh = c%2 (8 heads,
processed as 4 pairs of 2).  Host sums the two partial outputs per batch
and adds b_out_eff = b_out + b_v @ w_out (V-bias folded analytically).

v2 design (vs v1 baseline at ~489us):
  - x arrives PRE-TRANSPOSED from host (xT [D, T] bf16): no PE transposes.
  - All weights / Q/K/V / exp tiles in bf16 (matmul 1 cyc/col at any N).
  - Scores computed per head-PAIR with 2x row tiling: head A lives on SBUF
    partitions 0-63 of QT/KT, head B on 64-127; the two matmuls target
    different PSUM banks and run CONCURRENTLY on the PE (64x128 tiling).
  - One 2048-wide exp per 2-kt group covers both heads' scores -> ACT
    instruction overhead amortized (ACT is the roofline engine here:
    33.5M exps/core at 128/cycle @1.2GHz ~= 218us + overheads).
  - AV matmul flipped: stationary = exp tile [k128, q128] (fast bf16
    weight load), moving = [V | ones] [k128, 65].  The ones column makes
    the softmax denominator ride along as output column 64, and the
    output lands NATURAL [q, dh] so normalization is a per-partition
    reciprocal + tensor_scalar multiply on DVE (no partition broadcast).
  - OT transposed back via PE transposes (cheap, bf16) for the output
    projection; out projection accumulates the 4 pair-chunks in PSUM.
  - Software-pipelined emission: projection of pair p+1 and the output
    projection of earlier qw-blocks are interleaved into pair p's
    attention groups, so the ACT engine (the bottleneck) never waits.
"""

import os
import sys

sys.path.insert(0, "/opt/trn_rl_repo")
# The axon NTFF profiling hook is unavailable in this container; make sure a
# stray BASS_TRACE in the environment can't route us onto that path.
os.environ["BASS_NEVER_TRACE"] = "1"

import ml_dtypes
import numpy as np

import concourse.bass as bass
import concourse.mybir as mybir
import concourse.tile as tile
from concourse import bacc
from concourse.bass_utils import run_bass_kernel_spmd
from concourse.masks import make_identity

F32 = mybir.dt.float32
BF16 = mybir.dt.bfloat16
AF = mybir.ActivationFunctionType

T = 2048           # sequence length
D = 1024           # model dim
NH = 8             # heads per core
DH = 64            # head dim
DHC = NH * DH      # 512, head-slice width per core
NDK = D // 128     # 8 contraction tiles over d
NKT = T // 128     # 16 k tiles
NPAIR = 4          # head pairs per core
SCALE = 1.0 / np.sqrt(DH)
# Schraudolph fast-exp constants for the DVE-offloaded tiles (bf16
# bitpattern built in int16):  exp(s) ~= bitcast_bf16(int16(s*SA + SB)).
SEXP_A = float(SCALE * 128.0 / np.log(2.0))
SEXP_C = 5.57      # mantissa-linearization offset (min max-rel-err)
# every DVE_EVERYth kt-group's exp runs on DVE instead of ACT
DVE_EVERY = 4

_CACHED = {}


def _build_bass(phases=3, reps=1):
    nc = bacc.Bacc()
    xt_d = nc.dram_tensor("xt", [D, T], BF16, kind="ExternalInput")
    wqk_d = nc.dram_tensor("wqk", [D, 2 * DHC], BF16, kind="ExternalInput")
    wv_d = nc.dram_tensor("wv", [D, DHC], BF16, kind="ExternalInput")
    bqk_d = nc.dram_tensor("bqk", [128, 8], F32, kind="ExternalInput")
    wout_d = nc.dram_tensor("wout", [DHC, D], BF16, kind="ExternalInput")
    out_d = nc.dram_tensor("out", [T, D], F32, kind="ExternalOutput")

    with tile.TileContext(nc) as tc:
      for _rep in range(reps):
        with tc.tile_pool(name="const", bufs=1) as constp, \
             tc.tile_pool(name="wgt", bufs=1) as wgtp, \
             tc.tile_pool(name="ott", bufs=1) as ottp, \
             tc.tile_pool(name="qk", bufs=2) as qkp, \
             tc.tile_pool(name="vo", bufs=4) as vop, \
             tc.tile_pool(name="bc", bufs=2) as bcp, \
             tc.tile_pool(name="et", bufs=24) as etp, \
             tc.tile_pool(name="outs", bufs=3) as outsp, \
             tc.tile_pool(name="ps_s", bufs=2, space="PSUM") as pss, \
             tc.tile_pool(name="ps_o", bufs=1, space="PSUM") as pso, \
             tc.tile_pool(name="ps_p", bufs=2, space="PSUM") as psp:

            bqk_sb = constp.tile([128, 8], F32)
            nc.sync.dma_start(out=bqk_sb, in_=bqk_d[0:128, 0:8])

            # Persistent weight tiles, loaded once up front.
            xts = [wgtp.tile([128, T], BF16, tag=f"xt{dk}", name=f"xt{dk}")
                   for dk in range(NDK)]
            wqk = [wgtp.tile([128, 2 * DHC], BF16, tag=f"wqk{dk}",
                             name=f"wqk{dk}") for dk in range(NDK)]
            wv = [wgtp.tile([128, DHC], BF16, tag=f"wv{dk}", name=f"wv{dk}")
                  for dk in range(NDK)]
            wout = [wgtp.tile([128, D], BF16, tag=f"wo{i}", name=f"wo{i}")
                    for i in range(4)]
            for dk in range(NDK):
                nc.gpsimd.dma_start(out=wqk[dk],
                                    in_=wqk_d[dk * 128:(dk + 1) * 128, :])
                # alternate two DMA queues so xT lands in ~half the time
                eng = nc.sync if dk % 2 == 0 else nc.scalar
                eng.dma_start(out=xts[dk],
                              in_=xt_d[dk * 128:(dk + 1) * 128, :])
            for dk in range(NDK):
                nc.gpsimd.dma_start(out=wv[dk],
                                    in_=wv_d[dk * 128:(dk + 1) * 128, :])
            for i in range(4):
                nc.gpsimd.dma_start(out=wout[i],
                                    in_=wout_d[i * 128:(i + 1) * 128, :])

            OTT = [ottp.tile([128, T], BF16, tag=f"ott{p}", name=f"ott{p}")
                   for p in range(NPAIR)]

            # Per-pair rotating state
            state = {p: {} for p in range(NPAIR)}

            def proj_chunks(p):
                """Yield closures emitting pair p's projections (Q,K,V).
                V is projected two pairs at a time (even p only)."""
                st = state[p]

                def alloc():
                    st["QT"] = qkp.tile([128, T], BF16, tag="qt", name="qt")
                    st["KT"] = qkp.tile([128, T], BF16, tag="kt", name="kt")
                    if p % 2 == 0:
                        for pp in (p, p + 1):
                            vo = vop.tile([128, NKT, 2, DH + 1], BF16,
                                          tag="vo", name="vo")
                            state[pp]["VO"] = vo
                            nc.vector.memset(vo[:, :, :, DH:DH + 1], 1.0)
                yield alloc

                # Q^T / K^T: [dh-of-pair 128, T], bias folded via DVE.
                def qk_chunk(s, tw):
                    def emit():
                        pq = psp.tile([128, 512], F32, tag="prj", name="pq")
                        for dk in range(NDK):
                            nc.tensor.matmul(
                                pq,
                                wqk[dk][:, p * 256 + s * 128:
                                        p * 256 + (s + 1) * 128],
                                xts[dk][:, tw * 512:(tw + 1) * 512],
                                start=(dk == 0), stop=(dk == NDK - 1))
                        dest = st["KT"] if s else st["QT"]
                        nc.vector.tensor_scalar_add(
                            dest[:, tw * 512:(tw + 1) * 512], pq,
                            bqk_sb[:, p * 2 + s:p * 2 + s + 1])
                    return emit
                # K(tw0) and Q(tw0) first: window (p,0)'s first scores
                # need only these, so attention starts ~15us earlier.
                yield qk_chunk(1, 0)
                yield qk_chunk(0, 0)
                for tw in range(1, 4):
                    yield qk_chunk(1, tw)

                # V natural [t, 4*DH] per t-tile for two pairs at once;
                # no bias (host-folded).
                def v_chunk(tt):
                    def emit():
                        pv = psp.tile([128, 512], F32, tag="prj", name="pv")
                        for dk in range(NDK):
                            nc.tensor.matmul(
                                pv[:, 0:256],
                                xts[dk][:, tt * 128:(tt + 1) * 128],
                                wv[dk][:, p * 128:(p + 2) * 128],
                                start=(dk == 0), stop=(dk == NDK - 1))
                        for pp in (p, p + 1):
                            nc.vector.tensor_copy(
                                state[pp]["VO"][:, tt, :, 0:DH],
                                pv[:, (pp - p) * 128:(pp - p + 1) * 128]
                                .rearrange("p (h d) -> p h d", h=2))
                    return emit
                if p % 2 == 0:
                    for tt in range(NKT):
                        yield v_chunk(tt)
                for tw in range(1, 4):
                    yield qk_chunk(0, tw)

            def scores_exp_chunks(p, qw):
                """16 chunks, one per kt: row-tiled pair-concurrent scores
                + a 1024-wide exp covering both heads.  The [128,1024]
                scores tiles are double-buffered (2 banks x 2), so exp(g)
                and scores(g+1) overlap instead of strictly alternating.
                Returns (chunks, ets); ets[kt] is filled at emit time."""
                st = state[p]
                ets = [None] * NKT

                def one(g, on_dve):
                    def emit():
                        ps = pss.tile([128, 1024], F32, tag="s", name="ps_s")
                        # h=0 uses PE rows 0-63, h=1 rows 64-127 and a
                        # different PSUM bank: adjacent emission lets the
                        # two row tiles run concurrently.
                        for h in range(2):
                            nc.tensor.matmul(
                                ps[:, h * 512:(h + 1) * 512],
                                st["KT"][h * 64:(h + 1) * 64,
                                         g * 128:(g + 1) * 128],
                                st["QT"][h * 64:(h + 1) * 64,
                                         qw * 512:(qw + 1) * 512],
                                start=True, stop=True)
                        if on_dve:
                            # Schraudolph exp on DVE: one fused mult-add
                            # writing the bf16 bitpattern as int16
                            # (relieves ACT, the bottleneck engine).
                            eti = etp.tile([128, 1024], mybir.dt.int16,
                                           tag="et", name="et")
                            nc.vector.tensor_scalar(
                                eti, ps, SEXP_A, 127.0 * 128.0 - SEXP_C,
                                mybir.AluOpType.mult, mybir.AluOpType.add)
                            ets[g] = eti.bitcast(BF16)
                        else:
                            et = etp.tile([128, 1024], BF16, tag="et",
                                          name="et")
                            nc.scalar.activation(et, ps, AF.Exp, scale=SCALE)
                            ets[g] = et
                    return emit
                return [one(g, g % DVE_EVERY == DVE_EVERY - 1)
                        for g in range(NKT)], ets

            def av_start(p, h):
                """Allocate the [dh+1, 512] PSUM accumulator for one head
                (one start/stop group per bank slot -- PSUM zero regions
                are 2KB)."""
                return pso.tile([DH + 1, 512], F32, tag=f"ost{h}",
                                name=f"ost{h}")

            def av_quarter(p, h, ost, ets, quarter):
                """4 of the 16 V-stationary AV matmuls (N=512; the ones
                column in VO row 64 accumulates the softmax denominator)."""
                st = state[p]
                for kt in range(quarter * 4, quarter * 4 + 4):
                    nc.tensor.matmul(
                        ost,
                        st["VO"][:, kt, h, :],
                        ets[kt][:, h * 512:(h + 1) * 512],
                        start=(kt == 0), stop=(kt == NKT - 1))

            def av_norm(p, qw, h, ost):
                """OTT_h = ost[0:64] / ost[64] -- reciprocal, partition
                broadcast, multiply straight into the output-projection
                layout (no transposes needed in this orientation)."""
                rc = outsp.tile([1, 512], F32, tag="rc", name="rc")
                nc.vector.reciprocal(rc, ost[DH:DH + 1, :])
                bc = bcp.tile([DH, 512], F32, tag="bc", name="bc")
                nc.gpsimd.partition_broadcast(bc, rc)
                nc.vector.tensor_mul(
                    OTT[p][h * 64:(h + 1) * 64, qw * 512:(qw + 1) * 512],
                    ost[0:DH, :], bc)

            def outproj_chunk(tt, dc):
                def emit():
                    pf = psp.tile([128, 512], F32, tag="prj", name="pf")
                    for pp in range(NPAIR):
                        nc.tensor.matmul(
                            pf,
                            OTT[pp][:, tt * 128:(tt + 1) * 128],
                            wout[pp][:, dc * 512:(dc + 1) * 512],
                            start=(pp == 0), stop=(pp == NPAIR - 1))
                    osb = outsp.tile([128, 512], F32, tag="ob", name="osb")
                    nc.vector.tensor_copy(osb, pf)
                    eng = nc.sync if (tt + dc) % 2 == 0 else nc.scalar
                    eng.dma_start(
                        out=out_d[tt * 128:(tt + 1) * 128,
                                  dc * 512:(dc + 1) * 512],
                        in_=osb)
                return emit

            # ---- software-pipelined emission -------------------------
            # Steady state: the AV bursts of window (p, qw) interleave
            # 1:1 with the scores+exp chunks of the NEXT window, so the
            # exp engines always have work queued.  Projection of pair
            # p+1 rides along as low-priority filler; the output
            # projection runs per-qt as soon as pair 3 finishes a tile.
            # Prelude: pair-0 K/Q(tw0) interleave with window (0,0)'s
            # scores so exp starts as soon as the xT DMA lands.
            p0 = list(proj_chunks(0))
            se_chunks, cur_ets = scores_exp_chunks(0, 0)
            for ch in p0[0:3]:      # alloc, K tw0, Q tw0
                ch()
            for ch in se_chunks[0:4]:
                ch()
            se_chunks = se_chunks[4:]
            for ktw in p0[3:6]:     # K tw1..3, four score groups each
                ktw()
                for ch in se_chunks[0:4]:
                    ch()
                se_chunks = se_chunks[4:]
            for ch in p0[6:]:       # V 0..15, Q tw1..3
                ch()

            windows = [(p, qw) for p in range(NPAIR) for qw in range(4)]
            filler = []
            for wi, (p, qw) in enumerate(windows):
                for ch in se_chunks:    # leftovers (none in steady state)
                    ch()
                if wi + 1 < len(windows):
                    se_chunks, nxt_ets = scores_exp_chunks(*windows[wi + 1])
                else:
                    se_chunks, nxt_ets = [], None
                if qw == 0 and p + 1 < NPAIR:
                    filler.extend(proj_chunks(p + 1))
                for h in range(2):
                    ost = av_start(p, h)
                    for quarter in range(4):
                        av_quarter(p, h, ost, cur_ets, quarter)
                        for _ in range(2):
                            if se_chunks:
                                se_chunks.pop(0)()
                        if filler:
                            filler.pop(0)()
                    av_norm(p, qw, h, ost)
                if p == NPAIR - 1:
                    for tt in range(qw * 4, qw * 4 + 4):
                        for dc in range(2):
                            outproj_chunk(tt, dc)()
                cur_ets = nxt_ets
            while filler:
                filler.pop(0)()
    nc.compile()
    return nc


def _get_nc():
    if "nc" not in _CACHED:
        _CACHED["nc"] = _build_bass()
    return _CACHED["nc"]


def _shard_inputs(x, w_qkv, b_qkv, w_out):
    bf = ml_dtypes.bfloat16
    in_maps = []
    for c in range(8):
        b, hh = c // 2, c % 2
        xt = np.ascontiguousarray(x[b].T).astype(bf)
        # Q/K weights, pair-major: [D, (pair, {Q128, K128})]
        wqk = np.empty((D, 2 * DHC), dtype=np.float32)
        bqk = np.zeros((128, 8), dtype=np.float32)
        for p in range(NPAIR):
            cq = (hh * 8 + 2 * p) * 64
            wqk[:, p * 256:p * 256 + 128] = w_qkv[:, cq:cq + 128]
            wqk[:, p * 256 + 128:p * 256 + 256] = \
                w_qkv[:, D + cq:D + cq + 128]
            bqk[:, 2 * p] = b_qkv[cq:cq + 128]
            bqk[:, 2 * p + 1] = b_qkv[D + cq:D + cq + 128]
        wv = w_qkv[:, 2 * D + hh * DHC:2 * D + (hh + 1) * DHC]
        in_maps.append({
            "xt": xt,
            "wqk": wqk.astype(bf),
            "wv": np.ascontiguousarray(wv).astype(bf),
            "bqk": bqk,
            "wout": np.ascontiguousarray(
                w_out[hh * DHC:(hh + 1) * DHC, :]).astype(bf),
        })
    return in_maps


def run_sharded(x, w_qkv, b_qkv, w_out, b_out, trace=False, trace_kwargs=None):
    """Run the SPMD kernel; returns (full_output, BassKernelResults)."""
    x = np.asarray(x, dtype=np.float32)
    w_qkv = np.asarray(w_qkv, dtype=np.float32)
    b_qkv = np.asarray(b_qkv, dtype=np.float32)
    w_out = np.asarray(w_out, dtype=np.float32)
    b_out = np.asarray(b_out, dtype=np.float32)

    nc = _get_nc()
    in_maps = _shard_inputs(x, w_qkv, b_qkv, w_out)
    kwargs = {}
    if trace:
        kwargs["trace"] = True
        if trace_kwargs:
            kwargs["trace_kwargs"] = trace_kwargs
    res = run_bass_kernel_spmd(nc, in_maps, core_ids=list(range(8)), **kwargs)

    # V-bias contribution folded analytically: attn weights sum to 1, so
    # +b_v on V adds b_v @ w_out to every output row.
    b_out_eff = b_out + b_qkv[2 * D:] @ w_out
    B = x.shape[0]
    out = np.empty((B, T, D), dtype=np.float32)
    for b in range(B):
        out[b] = res.results[2 * b]["out"] + res.results[2 * b + 1]["out"] \
            + b_out_eff
    return out, res


def kernel(x, w_qkv, b_qkv, w_out, b_out):
    out, _ = run_sharded(x, w_qkv, b_qkv, w_out, b_out, trace=False)
    return out
